# revision 3
# baseline (speedup 1.0000x reference)
"""DayAdapter Trainium2 kernel.

y[b] = softsign(x[b] @ W[day_ids[b]] + b[day_ids[b]])
  x: [64, 1024, 512] f32, W: [24, 512, 512] f32, b: [24, 512] f32,
  day_ids: [64] i64.

Strategy: data-parallel over batch (8 samples per NeuronCore, 8 cores),
computing the TRANSPOSED output yT[e, t] = sum_d W[d, e] x[t, d] + b[e]:

  - Output partitions are the feature dim e, so the per-day bias is a
    per-partition scalar that folds into a single ACT op
    (Identity(acc + bias)) straight out of PSUM — no broadcast tile.
  - All wire traffic is bf16 (x, W in; y out, upcast on host), which
    halves HBM bytes vs f32: 20 MiB/core instead of 40 MiB. The
    correctness budget allows it (bf16 quantization gives ~3e-3 l2
    rel err vs the 2e-2 gate).
  - Host pre-arranges every tensor into the exact SBUF layout
    ([128 partitions, ...contiguous]) so every DMA is a full-contiguous
    8 KiB/partition transfer.
  - Softsign pipeline is spread across three engines so none exceeds
    the ~58-64 us DMA floor: ACT does the bias add (PSUM drain), DVE
    does den = |t|+1 (one dual-op tensor_scalar) and the fast
    reciprocal, GPSIMD does the final t * (1/den) multiply with the
    f32->bf16 downcast on write.

Per-core engine busy estimates: DMA ~58-64 us (bottleneck), PE ~55 us
(bf16 2 cols/cycle), ACT ~28 us, DVE ~34 us, GPSIMD ~28 us.
"""

import sys

if "/opt/trn_rl_repo" not in sys.path:
    sys.path.insert(0, "/opt/trn_rl_repo")

import numpy as np

import concourse.bacc as bacc
import concourse.mybir as mybir
import concourse.tile as tile
from concourse.bass_utils import run_bass_kernel_spmd

N_CORES = 8
B = 64
T = 1024
D = 512
S = B // N_CORES  # 8 samples per core
P = 128
KB = D // P  # 4 contraction blocks
EB = D // P  # 4 output-feature blocks
TB = T // 512  # 2 psum-width blocks of the t dim

_CACHE = {}

# test.py reads this for exec_time_ns after a traced run.
LAST_RESULTS = None
TRACE = False


def _build(bench_reps=None):
    key = ("prog", bench_reps)
    if key in _CACHE:
        return _CACHE[key]

    bf16 = mybir.dt.bfloat16
    f32 = mybir.dt.float32

    nc = bacc.Bacc("TRN2", debug=False, num_devices=N_CORES)

    xT = nc.dram_tensor("xT", [S, P, KB, T], bf16, kind="ExternalInput").ap()
    Wt = nc.dram_tensor("Wt", [S, P, KB, D], bf16, kind="ExternalInput").ap()
    bg = nc.dram_tensor("bg", [S, P, EB], f32, kind="ExternalInput").ap()
    y = nc.dram_tensor("y", [S, P, EB, T], bf16, kind="ExternalOutput").ap()

    with tile.TileContext(nc) as tc:
        with (
            tc.tile_pool(name="xt", bufs=3) as xt_pool,
            tc.tile_pool(name="w", bufs=3) as w_pool,
            tc.tile_pool(name="bias", bufs=3) as b_pool,
            tc.tile_pool(name="tt", bufs=3) as tt_pool,
            tc.tile_pool(name="work", bufs=3) as work_pool,
            tc.tile_pool(name="out", bufs=2) as out_pool,
            tc.tile_pool(name="psum", bufs=4, space="PSUM") as psum_pool,
        ):
            import contextlib

            loop_cm = (
                tc.For_i(
                    0,
                    bench_reps,
                    1,
                    hint_engines=(
                        mybir.EngineType.PE,
                        mybir.EngineType.Activation,
                        mybir.EngineType.DVE,
                        mybir.EngineType.SP,
                    ),
                )
                if bench_reps
                else contextlib.nullcontext()
            )
            with loop_cm:
                loaded = {}

                def load(s):
                    xs = xt_pool.tile([P, KB, T], bf16, tag="xs")
                    nc.sync.dma_start(xs[:], xT[s])
                    ws = w_pool.tile([P, KB, D], bf16, tag="ws")
                    nc.sync.dma_start(ws[:], Wt[s])
                    bs = b_pool.tile([P, EB], f32, tag="bs")
                    nc.sync.dma_start(bs[:], bg[s])
                    loaded[s] = (xs, ws, bs)

                load(0)
                if S > 1:
                    load(1)
                for s in range(S):
                    xs, ws, bs = loaded.pop(s)
                    outs = out_pool.tile([P, EB, T], bf16, tag="out")
                    for eb in range(EB):
                        if eb == 1 and s + 2 < S:
                            load(s + 2)
                        tt = tt_pool.tile([P, T], f32, tag="tt")
                        for tb in range(TB):
                            acc = psum_pool.tile([P, 512], f32, tag="acc")
                            for k in range(KB):
                                nc.tensor.matmul(
                                    acc[:],
                                    ws[:, k, eb * P : (eb + 1) * P],
                                    xs[:, k, tb * 512 : (tb + 1) * 512],
                                    start=(k == 0),
                                    stop=(k == KB - 1),
                                )
                            # ACT: tt = acc + bias (per-partition scalar)
                            nc.scalar.add(
                                tt[:, tb * 512 : (tb + 1) * 512],
                                acc[:],
                                bs[:, eb : eb + 1],
                            )
                        # DVE: ab = max(-tt, tt) = |tt|; den = ab + 1;
                        # rec = 1/den (~51 ULP)
                        ab = work_pool.tile([P, T], f32, tag="ab")
                        nc.vector.scalar_tensor_tensor(
                            ab[:],
                            tt[:],
                            -1.0,
                            tt[:],
                            mybir.AluOpType.mult,
                            mybir.AluOpType.max,
                        )
                        den = work_pool.tile([P, T], f32, tag="den")
                        nc.vector.tensor_scalar(
                            den[:], ab[:], 1.0, None, mybir.AluOpType.add
                        )
                        rec = work_pool.tile([P, T], f32, tag="rec")
                        nc.vector.reciprocal_approx_fast(rec[:], den[:])
                        # GPSIMD: out = tt * rec, downcast to bf16
                        nc.gpsimd.tensor_tensor(
                            outs[:, eb, :], tt[:], rec[:], mybir.AluOpType.mult
                        )
                    nc.gpsimd.dma_start(y[s], outs[:])

    nc.compile()
    _CACHE[key] = nc
    return nc


def _prepare_in_maps(x, day_ids, W, b):
    import ml_dtypes

    bf16 = ml_dtypes.bfloat16

    x = np.asarray(x, dtype=np.float32)
    W = np.asarray(W, dtype=np.float32)
    b = np.asarray(b, dtype=np.float32)
    ids = np.asarray(day_ids).astype(np.int64)

    # Host-side prep into exact SBUF layouts (partition dim first):
    #   xdev[s, p, k, t] = x[s, t, k*P + p]
    #   wdev[s, p, k, e] = W[ids[s], k*P + p, e]
    #   bdev[s, p, eb]   = b[ids[s], eb*P + p]
    xdev = np.ascontiguousarray(
        x.transpose(0, 2, 1).reshape(B, KB, P, T).transpose(0, 2, 1, 3)
    ).astype(bf16)
    wdev = np.ascontiguousarray(
        W[ids].reshape(B, KB, P, D).transpose(0, 2, 1, 3)
    ).astype(bf16)
    bdev = np.ascontiguousarray(b[ids].reshape(B, EB, P).transpose(0, 2, 1))

    in_maps = []
    for c in range(N_CORES):
        lo, hi = c * S, (c + 1) * S
        in_maps.append({"xT": xdev[lo:hi], "Wt": wdev[lo:hi], "bg": bdev[lo:hi]})
    return in_maps


def kernel(x, day_ids, W, b):
    global LAST_RESULTS
    in_maps = _prepare_in_maps(x, day_ids, W, b)
    nc = _build()
    res = run_bass_kernel_spmd(
        nc, in_maps, core_ids=list(range(N_CORES)), trace=TRACE
    )
    LAST_RESULTS = res
    # ydev[s, p, eb, t] = y[s, t, eb*P + p]
    ydev = np.concatenate(
        [res.results[c]["y"] for c in range(N_CORES)], axis=0
    )
    out = ydev.transpose(0, 3, 2, 1).reshape(B, T, D)
    return np.ascontiguousarray(out).astype(np.float32)


# revision 4
# speedup vs baseline: 1.8727x; 1.8727x over previous
"""DayAdapter Trainium2 kernel.

y[b] = softsign(x[b] @ W[day_ids[b]] + b[day_ids[b]])
  x: [64, 1024, 512] f32, W: [24, 512, 512] f32, b: [24, 512] f32,
  day_ids: [64] i64.

Strategy: data-parallel over batch (8 samples per NeuronCore, 8 cores),
computing the TRANSPOSED output yT[e, t] = sum_d W[d, e] x[t, d] + b[e]:

  - Output partitions are the feature dim e, so the per-day bias is a
    per-partition scalar that folds into a single ACT op
    (Identity(acc + bias)) straight out of PSUM — no broadcast tile.
  - All wire traffic is bf16 (x, W in; y out, upcast on host): 20 MiB
    per core instead of 40 MiB. Host pre-arranges every tensor into the
    exact SBUF layout so every DMA is a full-contiguous 8 KiB/partition
    transfer (~341 GB/s at 1 MiB).
  - Elementwise work is the real budget constraint (DVE f32 is only
    ~123 G elem/s; Pool mult ~64 G elem/s; ACT ~154 G elem/s), so the
    softsign is done in 3 passes over [128, 1024] tiles:
      ACT:  tt = acc + bias           (also drains the 2-bank PSUM pair)
      DVE:  rec = 1/(1 + |tt|)        (ONE fused 8-stage custom-DVE op:
                                       d = max(1+t, 1-t), BITWISE_NOT
                                       reciprocal seed + one Chebyshev-
                                       Newton pass, ~0.17% max rel err)
      DVE/Pool: out = tt * rec -> bf16 (multiply split between the two
                                       engines to balance busy time)
  - Output DMA rides ACT's hardware DGE queue; input loads ride SP's.

Per-core busy estimates: DMA ~60-66 us (bottleneck), PE ~55 us (bf16),
DVE ~50 us, Pool ~50 us, ACT ~40 us.
"""

import sys

if "/opt/trn_rl_repo" not in sys.path:
    sys.path.insert(0, "/opt/trn_rl_repo")

import numpy as np

import concourse.bacc as bacc
import concourse.mybir as mybir
import concourse.tile as tile
from concourse.bass_utils import run_bass_kernel_spmd

N_CORES = 8
B = 64
T = 1024
D = 512
S = B // N_CORES  # 8 samples per core
P = 128
KB = D // P  # 4 contraction blocks
EB = D // P  # 4 output-feature blocks
TB = T // 512  # 2 psum-bank blocks of the t dim

# ---------------------------------------------------------------------------
# Custom fused DVE op: rec = 1/(1 + |t|) in one 8-stage pass.
#   d  = max(1+t, 1-t) = 1 + |t|          (3 stages)
#   y0 = bitcast(~d) * C0                 (2 stages: reciprocal seed)
#   y1 = y0 * (C1 - d*y0)                 (3 stages: Chebyshev-Newton)
# Uses the RECIPROCAL_APPROX_FAST constants; dropping its second Newton
# pass costs ~0.17% max rel err, far inside the accuracy budget.
# ---------------------------------------------------------------------------
from concourse import dve_ops
from concourse.dve_spec import (
    AluOp,
    Bin,
    One,
    Spec,
    Src0,
    lower,
    maxx,
    _has_src1,
)
from concourse.dve_uop import DveOpSpec


def _ref_softsign_recip(in0, in1, s0, s1, imm2):
    d = (1.0 + np.abs(in0)).astype(np.float32)
    not_d = (~d.view(np.int32)).view(np.float32)
    y0 = not_d * np.float32(s0)
    return y0 * (np.float32(s1) - d * y0)


def _register_softsign_recip():
    name = "SOFTSIGN_RECIP_1P_ANT"
    if name in dve_ops._SUB_OPCODE_FOR_NAME:
        for op in dve_ops.OPS:
            if op.name == name:
                return op
    _d = maxx(One + Src0, One - Src0)
    _nd = Bin(AluOp.BITWISE_NOT, _d, _d)
    _y0 = _nd * dve_ops.C0
    body = _y0 * (dve_ops.C1 - _d * _y0)
    spec = Spec(body=body, reference=_ref_softsign_recip)
    row = dve_ops._CUSTOM_DVE_ROW_BASE + len(dve_ops.OPS)
    dve_ops._SUB_OPCODE_FOR_NAME[name] = row
    shas = {}
    for ver in ("v3", "v4"):
        tmp = DveOpSpec(
            name=name, opcode=row, uops=lower(spec, ver=ver),
            rd1_en=_has_src1(spec),
        )
        shas[ver] = tmp.sha(ver)
    op = dve_ops.DveOp(name, spec, subdim=False, uops_sha=shas)
    dve_ops.OPS.append(op)
    dve_ops.CUSTOM_DVE_SPECS[name] = spec
    return op


SOFTSIGN_RECIP_1P = _register_softsign_recip()
_RECIP_C = dve_ops.RECIP_APPROX_FAST_CONSTS

_CACHE = {}

# test.py reads this for exec_time_ns after a traced run.
LAST_RESULTS = None
TRACE = False

# Which eb-groups compute the final multiply on DVE (rest go to Pool).
DVE_MUL_EBS = (0,)


def _build(bench_reps=None):
    key = ("prog", bench_reps, DVE_MUL_EBS)
    if key in _CACHE:
        return _CACHE[key]

    bf16 = mybir.dt.bfloat16
    f32 = mybir.dt.float32

    nc = bacc.Bacc("TRN2", debug=False, num_devices=N_CORES)

    xT = nc.dram_tensor("xT", [S, P, KB, T], bf16, kind="ExternalInput").ap()
    Wt = nc.dram_tensor("Wt", [S, P, KB, D], bf16, kind="ExternalInput").ap()
    bg = nc.dram_tensor("bg", [S, P, EB], f32, kind="ExternalInput").ap()
    y = nc.dram_tensor("y", [S, P, EB, T], bf16, kind="ExternalOutput").ap()

    with tile.TileContext(nc) as tc:
        with (
            tc.tile_pool(name="xt", bufs=3) as xt_pool,
            tc.tile_pool(name="w", bufs=3) as w_pool,
            tc.tile_pool(name="bias", bufs=3) as b_pool,
            tc.tile_pool(name="tt", bufs=3) as tt_pool,
            tc.tile_pool(name="work", bufs=3) as work_pool,
            tc.tile_pool(name="out", bufs=2) as out_pool,
            tc.tile_pool(name="psum", bufs=3, space="PSUM") as psum_pool,
        ):
            import contextlib

            loop_cm = (
                tc.For_i(
                    0,
                    bench_reps,
                    1,
                    hint_engines=(
                        mybir.EngineType.PE,
                        mybir.EngineType.Activation,
                        mybir.EngineType.DVE,
                        mybir.EngineType.SP,
                    ),
                )
                if bench_reps
                else contextlib.nullcontext()
            )
            with loop_cm:
                loaded = {}

                def load(s):
                    xs = xt_pool.tile([P, KB, T], bf16, tag="xs")
                    nc.sync.dma_start(xs[:], xT[s])
                    ws = w_pool.tile([P, KB, D], bf16, tag="ws")
                    nc.sync.dma_start(ws[:], Wt[s])
                    bs = b_pool.tile([P, EB], f32, tag="bs")
                    nc.sync.dma_start(bs[:], bg[s])
                    loaded[s] = (xs, ws, bs)

                load(0)
                if S > 1:
                    load(1)
                for s in range(S):
                    xs, ws, bs = loaded.pop(s)
                    outs = out_pool.tile([P, EB, T], bf16, tag="out")
                    for eb in range(EB):
                        if eb == 1 and s + 2 < S:
                            load(s + 2)
                        acc = psum_pool.tile([P, T], f32, tag="acc")
                        for tb in range(TB):
                            for k in range(KB):
                                nc.tensor.matmul(
                                    acc[:, tb * 512 : (tb + 1) * 512],
                                    ws[:, k, eb * P : (eb + 1) * P],
                                    xs[:, k, tb * 512 : (tb + 1) * 512],
                                    start=(k == 0),
                                    stop=(k == KB - 1),
                                )
                        # ACT: tt = acc + bias (per-partition scalar),
                        # drains both PSUM banks in one op
                        tt = tt_pool.tile([P, T], f32, tag="tt")
                        nc.scalar.add(tt[:], acc[:], bs[:, eb : eb + 1])
                        # DVE: rec = 1/(1 + |tt|), one fused pass
                        rec = work_pool.tile([P, T], f32, tag="rec")
                        nc.vector._custom_dve(
                            SOFTSIGN_RECIP_1P,
                            out=rec[:],
                            in0=tt[:],
                            s0=_RECIP_C["s0"],
                            s1=_RECIP_C["s1"],
                        )
                        # final multiply, split DVE/Pool for balance
                        eng = (
                            nc.vector if eb in DVE_MUL_EBS else nc.gpsimd
                        )
                        eng.tensor_tensor(
                            outs[:, eb, :], tt[:], rec[:],
                            mybir.AluOpType.mult,
                        )
                    nc.scalar.dma_start(y[s], outs[:])

    nc.compile()
    _CACHE[key] = nc
    return nc


def _prepare_in_maps(x, day_ids, W, b):
    import ml_dtypes

    bf16 = ml_dtypes.bfloat16

    x = np.asarray(x, dtype=np.float32)
    W = np.asarray(W, dtype=np.float32)
    b = np.asarray(b, dtype=np.float32)
    ids = np.asarray(day_ids).astype(np.int64)

    # Host-side prep into exact SBUF layouts (partition dim first):
    #   xdev[s, p, k, t] = x[s, t, k*P + p]
    #   wdev[s, p, k, e] = W[ids[s], k*P + p, e]
    #   bdev[s, p, eb]   = b[ids[s], eb*P + p]
    xdev = np.ascontiguousarray(
        x.transpose(0, 2, 1).reshape(B, KB, P, T).transpose(0, 2, 1, 3)
    ).astype(bf16)
    wdev = np.ascontiguousarray(
        W[ids].reshape(B, KB, P, D).transpose(0, 2, 1, 3)
    ).astype(bf16)
    bdev = np.ascontiguousarray(b[ids].reshape(B, EB, P).transpose(0, 2, 1))

    in_maps = []
    for c in range(N_CORES):
        lo, hi = c * S, (c + 1) * S
        in_maps.append({"xT": xdev[lo:hi], "Wt": wdev[lo:hi], "bg": bdev[lo:hi]})
    return in_maps


def kernel(x, day_ids, W, b):
    global LAST_RESULTS
    in_maps = _prepare_in_maps(x, day_ids, W, b)
    nc = _build()
    res = run_bass_kernel_spmd(
        nc, in_maps, core_ids=list(range(N_CORES)), trace=TRACE
    )
    LAST_RESULTS = res
    # ydev[s, p, eb, t] = y[s, t, eb*P + p]
    ydev = np.concatenate(
        [res.results[c]["y"] for c in range(N_CORES)], axis=0
    )
    out = ydev.transpose(0, 3, 2, 1).reshape(B, T, D)
    return np.ascontiguousarray(out).astype(np.float32)
